# revision 30
# baseline (speedup 1.0000x reference)
"""Trainium2 Bass kernel for the Expected-Depth DP loss.

Computation (see reference):
  - edge_max = max over first 7 of 8 op-logits          [S, 64, 16]
  - w        = masked softmax over the 16-wide window   [S, 64, 16]
  - DP scan:  ed[j] = sum_k w[j,k] * (ed[base+k] + 1),  j = 2..65
  - loss     = sum_s theta[s] * softmax(beta[s]) . (ed[ii] + ed[jj])

Sharding: S=8192 stages split across 8 cores (pure data parallel,
1024 stages/core). Per-core partial losses are summed on the host.

v3 structure:
  - alpha is host-transposed into 8 node-slabs (slab i = nodes
    [8i,8i+8) of all 1024 stages; each partition reads one contiguous
    32KB chunk) and streamed fp32 over the sync HWDGE queue; the last
    slab is split into 4 quarter-DMAs so its processing tail is short.
  - after slab i is reduced to softmax weights, DP steps 8i+2..8i+9
    run over all 8 stage-tiles, so the sequential scan streams along
    with the DMA instead of trailing it.
  - beta is cast fp32->bf16 during gpsimd SWDGE DMAs, paced one tile
    per slab by the gpsimd program order so it steals little alpha
    bandwidth; exp(beta) lands in one persistent SBUF buffer.
  - the incidence reduction runs with mt as the (shared) PE weights:
    per chunk one LDWEIGHTS feeds a 512-column matmul over 4 stage-
    tiles, accumulating c^T [67, 512] in PSUM (2 quad passes).  c^T is
    transposed back per-tile by PE transposes, then theta/denominator
    are folded into c before the DP finishes, leaving one fused
    dot + partition reduction as the only post-DP work.
"""

import numpy as np

SW = 16          # DP window
NN = 64          # nodes per stage
NOPS = 8         # ops per edge (last excluded from the max)
S = 8192         # stages
E = 2016         # beta edges
P = 128          # SBUF partitions
N_CORES = 8
S_CORE = S // N_CORES        # 1024
T = S_CORE // P              # 8 stage-tiles per core
NSLAB = 8                    # node slabs
NPS = NN // NSLAB            # 8 nodes per slab
SLABW = T * NPS * SW         # 1024 mx elems per slab (per partition)
AWS_ = SLABW * NOPS          # 8192 alpha floats per slab per partition
NQ = 4                       # quarters for the last slab
QW = AWS_ // NQ              # 2048 floats per quarter
EDW = 67                     # ed row stride (66 node slots + 1 pad)
NCH = 16                     # beta column chunks
ECH = E // NCH               # 126 edges per chunk
MNEG = -40.0                 # additive mask for invalid window rows

_CACHE = {}


def _host_consts():
    import ml_dtypes

    ii, jj = [], []
    for i in range(2, NN + 1):
        for j in range(i + 1, NN + 2):
            ii.append(i)
            jj.append(j)
    ii = np.asarray(ii)
    jj = np.asarray(jj)
    # incidence matrix chunks: mt[e_local, c*67 + k] = [ii==k] + [jj==k],
    # column 66 of each chunk is all ones (softmax denominator)
    mt = np.zeros((NCH, ECH, EDW), np.float32)
    for e in range(E):
        c, el = divmod(e, ECH)
        mt[c, el, ii[e]] += 1.0
        mt[c, el, jj[e]] += 1.0
        mt[c, el, EDW - 1] = 1.0
    mt = np.ascontiguousarray(
        mt.transpose(1, 0, 2).reshape(ECH, NCH * EDW)
    ).astype(ml_dtypes.bfloat16)

    # additive masks for slabs 0/1 (node n: rows k < n+2 valid);
    # slab layout is node-major [nl, t, k]
    def neg(nodes):
        m = np.zeros((len(nodes), T, SW), np.float32)
        for r, n in enumerate(nodes):
            m[r, :, min(n + 2, SW):] = MNEG
        return np.ascontiguousarray(
            np.broadcast_to(m.reshape(1, -1), (P, m.size))
        ).astype(ml_dtypes.bfloat16)

    mneg0 = neg(range(0, NPS))          # slab 0: nodes 0-7   [P, 1024]
    mneg1 = neg(range(NPS, 14))         # slab 1: nodes 8-13  [P, 768]
    ident = np.eye(P, dtype=np.float32)
    return mt, mneg0, mneg1, ident


def _build_nc():
    import concourse.bass as bass
    import concourse.mybir as mybir
    from concourse.tile import TileContext
    from concourse.vector_clock import ScopedClock, VectorClock

    # This walrus build rejects TPB instructions carrying more than one sem
    # wait (two for EventSemaphore), but Tile's wait assignment happily packs
    # 2-3. Split the extras onto single-wait NoOps on the same engine.
    if not getattr(TileContext, "_ant_wait_split", False):
        _orig_commit = TileContext._commit_instruction

        def _commit_split(self, inst, lazy_reg_writes=True):
            si = inst.sync_info
            limit = 2 if isinstance(inst, mybir.InstEventSemaphore) else 1
            if si is not None and si.on_wait and len(si.on_wait) > limit:
                waits = list(si.on_wait)
                for i, w in enumerate(waits[:-limit]):
                    nop = mybir.InstNoOp(
                        name=f"{inst.name}-sw{i}",
                        sync_info=mybir.SyncInfo(on_wait=[w], on_update=[]),
                        bass_nofuse=True,
                        engine=inst.engine,
                    )
                    _orig_commit(self, nop, lazy_reg_writes)
                inst.sync_info = mybir.SyncInfo(
                    on_wait=waits[-limit:], on_update=list(si.on_update)
                )
            return _orig_commit(self, inst, lazy_reg_writes)

        TileContext._commit_instruction = _commit_split
        TileContext._ant_wait_split = True

    # The stock TileContext tail drain packs every outstanding sem wait into
    # a single InstDrain; this walrus caps non-EventSemaphore instructions at
    # one wait. Emit one drain per outstanding semaphore instead.
    def _drain_and_barrier(self, tick_clock, wait_clock):
        nc = self.nc
        gc = tick_clock.global_clock
        n = len(gc)
        for i in range(n):
            t = gc[i]
            if t <= 0:
                continue
            vc = VectorClock([0] * n)
            vc.require_at_least(i, t)
            d = nc.sync.drain()
            wait_clock.add_sem_waits(d.ins, ScopedClock({None: vc}))
        nc.all_engine_barrier()
        assert self.sems is not None
        popped = nc._tile_sem_poison_stack.pop()
        assert popped is self._sem_poison
        nc.clear_and_free_semaphores(list(self.sems.allocated().values()))
        nc.all_engine_barrier()

    TileContext._drain_and_barrier = _drain_and_barrier

    f32 = mybir.dt.float32
    bf16 = mybir.dt.bfloat16
    Alu = mybir.AluOpType
    Act = mybir.ActivationFunctionType
    X = mybir.AxisListType.X

    nc = bass.Bass()
    # alpha host-transposed to slab-major: [slab, p, t, nl, k, o] flattened
    # to [NSLAB*P, 8192]; partition p's slab read is 32KB contiguous
    alpha_d = nc.declare_dram_parameter(
        "alpha_s", [NSLAB * P, AWS_], f32, isOutput=False
    )
    # beta pre-transposed on the host into chunk layout:
    # beta_t[el, t*2048 + c*128 + p] = beta[t*128 + p, c*126 + el]
    beta_d = nc.declare_dram_parameter("beta_t", [ECH, T * NCH * P], f32, isOutput=False)
    theta_d = nc.declare_dram_parameter("theta_t", [P, T], f32, isOutput=False)
    mneg0_d = nc.declare_dram_parameter("mneg0", [P, NPS * T * SW], bf16, isOutput=False)
    mneg1_d = nc.declare_dram_parameter("mneg1", [P, 6 * T * SW], bf16, isOutput=False)
    mt_d = nc.declare_dram_parameter("mt_c", [ECH, NCH * EDW], bf16, isOutput=False)
    ident_d = nc.declare_dram_parameter("ident", [P, P], f32, isOutput=False)
    out_d = nc.declare_dram_parameter("loss_part", [1, 1], f32, isOutput=True)

    with TileContext(nc) as tc:
        with (
            tc.tile_pool(name="consts", bufs=1) as cp,
            tc.tile_pool(name="alphap", bufs=3) as ap_pool,
            tc.tile_pool(name="mxp", bufs=2) as mxp,
            tc.tile_pool(name="persist", bufs=1) as pp,
            tc.tile_pool(name="smallp", bufs=4) as sp,
            tc.tile_pool(name="betap", bufs=4) as bp,
            tc.tile_pool(name="psc", bufs=2, space="PSUM") as psc,
        ):
            # consts first (small; bf16 masks) so alpha starts right after
            mneg0_sb = cp.tile([P, NPS * T * SW], bf16)
            nc.sync.dma_start(mneg0_sb[:, :], mneg0_d[:, :])
            mneg1_sb = cp.tile([P, 6 * T * SW], bf16)
            nc.sync.dma_start(mneg1_sb[:, :], mneg1_d[:, :])
            mt_sb = cp.tile([ECH, NCH * EDW], bf16)
            nc.sync.dma_start(mt_sb[:, :], mt_d[:, :])
            theta_sb = cp.tile([P, T], f32)
            nc.sync.dma_start(theta_sb[:, :], theta_d[:, :])
            ident_sb = cp.tile([P, P], f32)
            nc.sync.dma_start(ident_sb[:, :], ident_d[:, :])

            # ---- alpha DMAs on the sync HWDGE queue; slabs 0 and 7 are
            # quarter-split (0: earlier compute start; 7: short tail) ----
            a_sl = []
            for i in range(NSLAB):
                a = ap_pool.tile([P, AWS_], f32, tag="a")
                if i in (0, NSLAB - 1):
                    for q in range(NQ):
                        nc.sync.dma_start(
                            a[:, q * QW : (q + 1) * QW],
                            alpha_d[i * P : (i + 1) * P, q * QW : (q + 1) * QW],
                        )
                else:
                    nc.sync.dma_start(a[:, :], alpha_d[i * P : (i + 1) * P, :])
                a_sl.append(a)
            ones_sb = cp.tile([P, 1], f32)
            nc.vector.memset(ones_sb[:, :], 1.0)

            w_sb = pp.tile([P, NSLAB * SLABW], f32)  # weights, slab-major
            ed_sb = pp.tile([P, T * EDW], f32)       # DP state, zero-init
            tmp_sb = pp.tile([P, T * (SW + 1)], f32) # DP scratch; col 0 == 1.0
            eb_sb = pp.tile([ECH, T * NCH * P], bf16)  # exp(beta), all tiles
            ct_sb = pp.tile([EDW, T * P], f32)       # c^T quads (PSUM copies)
            c_sb = pp.tile([P, T * EDW], f32)        # c per stage-partition
            cw_sb = pp.tile([P, T * (EDW - 1)], f32) # c * theta / den
            nc.vector.memset(ed_sb[:, :], 0.0)

            ed3 = ed_sb.rearrange("p (t k) -> p t k", t=T)
            tmp3 = tmp_sb.rearrange("p (t k) -> p t k", k=SW + 1)
            nc.vector.memset(tmp3[:, :, 0:1], 1.0)
            w5 = w_sb.rearrange(
                "p (s n t k) -> p s n t k", s=NSLAB, n=NPS, k=SW
            )

            def dp_steps(j_lo, j_hi, fill=()):
                """DP steps with the multiply on gpsimd and the grouped
                reduce on DVE; `fill` supplies independent DVE thunks that
                are interleaved so DVE stays busy during the gp multiply."""
                fill = list(fill)
                for j in range(j_lo, j_hi):
                    n = j - 2
                    wid = min(j, SW)
                    base = j - wid
                    # sum(w) == 1, so ed[j] = 1 + sum(w * ed); the constant
                    # 1.0 lives in tmp col 0 and rides along in the reduce.
                    # Products of the settled window prefix run on gpsimd a
                    # full step ahead; DVE multiplies only ed[j-1].
                    nc.gpsimd.tensor_mul(
                        tmp3[:, :, 1:wid],
                        ed3[:, :, base : j - 1],
                        w5[:, n // NPS, n % NPS, :, 0 : wid - 1],
                    )
                    nc.vector.tensor_mul(
                        tmp3[:, :, wid : wid + 1],
                        ed3[:, :, j - 1 : j],
                        w5[:, n // NPS, n % NPS, :, wid - 1 : wid],
                    )
                    if fill:
                        fill.pop(0)()
                    nc.vector.reduce_sum(
                        ed3[:, :, j : j + 1], tmp3[:, :, 0 : wid + 1], axis=X
                    )
                for f in fill:
                    f()

            def slab_front(i, a_view, ncols, coff, nchunk=1):
                """Thunks for max-reduce + mask + exp + sums of `ncols` mx
                columns of slab i at column offset coff, split into `nchunk`
                pieces.  Returns (dve_thunks, norm_fn); the thunks interleave
                with a DP burst to keep DVE busy during the gp multiplies,
                and norm_fn is emitted after the burst so it never stalls
                the burst's gp stream."""
                a3 = a_view.rearrange("p (g o) -> p g o", o=NOPS)
                mx = mxp.tile([P, SLABW], f32, tag="mx", bufs=2)
                mxc = mx[:, coff : coff + ncols]
                e_sl = w_sb[:, i * SLABW + coff : i * SLABW + coff + ncols]
                ng = ncols // SW
                ngc = ng // nchunk
                gper = ngc * SW
                s_t = sp.tile([P, T * NPS], f32, tag="s")
                rs = sp.tile([P, T * NPS], f32, tag="rs")
                mn = (mneg0_sb if i == 0 else mneg1_sb) if i < 2 else None

                def front(c):
                    nc.vector.reduce_max(
                        mxc[:, c * gper : (c + 1) * gper],
                        a3[:, c * gper : (c + 1) * gper, 0 : NOPS - 1],
                        axis=X,
                    )
                    if mn is not None:
                        lo, hi = c * gper, min((c + 1) * gper, mn.shape[1])
                        if lo < hi:
                            nc.vector.tensor_add(
                                mx[:, lo:hi], mx[:, lo:hi], mn[:, lo:hi]
                            )
                    nc.scalar.activation(
                        e_sl[:, c * gper : (c + 1) * gper],
                        mxc[:, c * gper : (c + 1) * gper],
                        Act.Exp,
                    )

                def sums(c):
                    nc.vector.reduce_sum(
                        s_t[:, c * ngc : (c + 1) * ngc],
                        e_sl[:, c * gper : (c + 1) * gper].rearrange(
                            "p (n k) -> p n k", k=SW
                        ),
                        axis=X,
                    )
                    if c == nchunk - 1:
                        nc.vector.reciprocal(rs[:, 0:ng], s_t[:, 0:ng])

                thunks = [lambda c=c: front(c) for c in range(nchunk)]
                thunks += [lambda c=c: sums(c) for c in range(nchunk)]

                def norm_fn():
                    rs_b = rs[:, 0:ng].rearrange(
                        "p (n o) -> p n o", o=1
                    ).broadcast_to((P, ng, SW))
                    e3 = e_sl.rearrange("p (n k) -> p n k", k=SW)
                    nc.gpsimd.tensor_mul(e3, e3, rs_b)

                return thunks, norm_fn

            # beta incidence matmuls: mt as shared weights, quad passes.
            # quad h covers stage-tiles 4h..4h+3; out is c^T [67, 512].
            ebv = eb_sb.rearrange("e (t c p) -> e t c p", t=T, p=P)
            ct_ps = psc.tile([P, T * P], f32, tag="ct", bufs=1)

            def beta_quad(h):
                cps = psc.tile([EDW, 4 * P], f32, tag="cq")
                for c in range(NCH):
                    nc.tensor.matmul(
                        cps[:, :],
                        mt_sb[:, c * EDW : (c + 1) * EDW],
                        ebv[:, 4 * h : 4 * h + 4, c, :],
                        start=(c == 0),
                        stop=(c == NCH - 1),
                    )
                nc.scalar.copy(ct_sb[:, h * 4 * P : (h + 1) * 4 * P], cps[:, :])
                for t in range(4 * h, 4 * h + 4):
                    nc.tensor.transpose(
                        ct_ps[:, t * P : t * P + EDW],
                        ct_sb[:, t * P : (t + 1) * P],
                        ident_sb[0:EDW, 0:EDW],
                    )
                    nc.scalar.copy(
                        c_sb[:, t * EDW : (t + 1) * EDW],
                        ct_ps[:, t * P : t * P + EDW],
                    )

            # beta DMA triggers paced ~one per slab on the gp SWDGE queue;
            # exp(beta) trails its DMA by one slab so ACT never stalls on it
            B_TRIG = {0: (0, 1), 1: (2,), 2: (3,), 3: (4,), 4: (5,), 5: (6, 7)}
            B_EXP = {1: (0, 1), 2: (2,), 3: (3,), 4: (4,), 5: (5,), 6: (6, 7)}
            b_t = {}

            # ---- slab loop (slabs 0-6 whole; slab 7 per quarter) ----
            # tile_wait_until pins each region's scheduler dispatch time so
            # the list scheduler cannot hoist late-phase ops (whose sems
            # would then head-block an engine's in-order stream)
            for i in range(NSLAB - 1):
                with tc.tile_wait_until(0.012 + 0.0122 * i):
                    th, norm_fn = slab_front(
                        i, a_sl[i], SLABW, 0, nchunk=(4 if i == 0 else 1)
                    )
                    for f in th:
                        f()
                    norm_fn()
                    for t in B_TRIG.get(i, ()):
                        b = bp.tile([ECH, NCH * P], bf16, tag="b")
                        nc.gpsimd.dma_start(
                            b[:, :], beta_d[:, t * NCH * P : (t + 1) * NCH * P]
                        )
                        b_t[t] = b
                    for t in B_EXP.get(i, ()):
                        nc.scalar.activation(
                            eb_sb[:, t * NCH * P : (t + 1) * NCH * P],
                            b_t.pop(t)[:, :],
                            Act.Exp,
                        )
                    if i > 0:
                        dp_steps(2 + (i - 1) * NPS, 2 + i * NPS)
                    if i == 4:
                        beta_quad(0)

            # slab 7 quarters: 2 nodes each; DP trails by one quarter
            i7 = NSLAB - 1
            a7 = a_sl[i7]
            for q in range(NQ):
                with tc.tile_wait_until(0.102 + 0.004 * q):
                    qv = a7[:, q * QW : (q + 1) * QW]
                    th, norm_fn = slab_front(
                        i7, qv, QW // NOPS, q * (SLABW // NQ)
                    )
                    for f in th:
                        f()
                    norm_fn()
                    if q == 0:
                        dp_steps(2 + (i7 - 1) * NPS, 2 + i7 * NPS)
                        beta_quad(1)
                    else:
                        dp_steps(
                            2 + i7 * NPS + (q - 1) * 2, 2 + i7 * NPS + q * 2
                        )

            # fold theta/denominator into c while the DP finishes
            with tc.tile_wait_until(0.124):
                c3 = c_sb.rearrange("p (t k) -> p t k", t=T)
                den = sp.tile([P, T], f32, tag="den")
                nc.vector.reciprocal(den[:, :], c3[:, :, EDW - 1])
                thr = sp.tile([P, T], f32, tag="thr")
                nc.vector.tensor_mul(thr[:, :], den[:, :], theta_sb[:, :])
                cw3 = cw_sb.rearrange("p (t k) -> p t k", t=T)
                nc.vector.tensor_mul(
                    cw3,
                    c3[:, :, 0 : EDW - 1],
                    thr.rearrange("p (t o) -> p t o", o=1).broadcast_to(
                        (P, T, EDW - 1)
                    ),
                )
                dp_steps(NN, NN + 2)

            # ---- post-DP: one fused dot + partition reduction ----
            with tc.tile_wait_until(0.127):
                prod = pp.tile([P, T * (EDW - 1)], f32)
                lrow = sp.tile([P, 1], f32, tag="lrow")
                nc.vector.scalar_tensor_tensor(
                    prod[:, :],
                    ed3[:, :, 0 : EDW - 1],
                    0.0,
                    cw3,
                    Alu.add,
                    Alu.mult,
                    accum_out=lrow[:, :],
                )
                out_ps = psc.tile([1, 1], f32, tag="outp", bufs=1)
                nc.tensor.matmul(
                    out_ps[:, :], lrow[:, :], ones_sb[:, :], start=True, stop=True
                )
                out_sb = sp.tile([1, 1], f32, tag="outs")
                nc.scalar.copy(out_sb[:, :], out_ps[:, :])
                nc.sync.dma_start(out_d[:, :], out_sb[:, :])

    return nc


def _get_compiled():
    if "nc" not in _CACHE:
        _CACHE["nc"] = _build_nc()
        _CACHE["consts"] = _host_consts()
    return _CACHE["nc"], _CACHE["consts"]


def _in_maps(alpha, beta, theta):
    mt, mneg0, mneg1, ident = _get_compiled()[1]
    alpha = np.ascontiguousarray(alpha, dtype=np.float32).reshape(S, NN * SW * NOPS)
    beta = np.ascontiguousarray(beta, dtype=np.float32)
    theta = np.ascontiguousarray(theta, dtype=np.float32)
    maps = []
    for c in range(N_CORES):
        sl = slice(c * S_CORE, (c + 1) * S_CORE)
        # [slab, p, nl, t, k*o]: partition p reads 32KB contiguous per
        # slab; node-major within the slab so quarter-DMAs cover node
        # prefixes for all stage-tiles
        alpha_s = np.ascontiguousarray(
            alpha[sl]
            .reshape(T, P, NSLAB, NPS, SW * NOPS)
            .transpose(2, 1, 3, 0, 4)
            .reshape(NSLAB * P, AWS_)
        )
        # [el, t*2048 + ch*128 + p] = beta[t*128 + p, ch*126 + el]
        beta_t = np.ascontiguousarray(
            beta[sl].reshape(T, P, NCH, ECH).transpose(3, 0, 2, 1).reshape(ECH, -1)
        )
        maps.append(
            {
                "alpha_s": alpha_s,
                "beta_t": beta_t,
                "theta_t": np.ascontiguousarray(theta[sl].reshape(T, P).T),
                "mneg0": mneg0,
                "mneg1": mneg1,
                "mt_c": mt,
                "ident": ident,
            }
        )
    return maps


def _run(alpha, beta, theta, **spmd_kwargs):
    from concourse.bass_utils import run_bass_kernel_spmd

    nc, _ = _get_compiled()
    res = run_bass_kernel_spmd(
        nc, _in_maps(alpha, beta, theta), core_ids=list(range(N_CORES)), **spmd_kwargs
    )
    total = np.float32(0.0)
    for r in res.results:
        total += np.float32(r["loss_part"][0, 0])
    return np.float32(total), res


def kernel(alpha, beta, theta):
    out, _ = _run(alpha, beta, theta)
    return out


# revision 31
# speedup vs baseline: 1.0728x; 1.0728x over previous
"""Trainium2 Bass kernel for the Expected-Depth DP loss.

Computation (see reference):
  - edge_max = max over first 7 of 8 op-logits          [S, 64, 16]
  - w        = masked softmax over the 16-wide window   [S, 64, 16]
  - DP scan:  ed[j] = sum_k w[j,k] * (ed[base+k] + 1),  j = 2..65
  - loss     = sum_s theta[s] * softmax(beta[s]) . (ed[ii] + ed[jj])

Sharding: S=8192 stages split across 8 cores (pure data parallel,
1024 stages/core). Per-core partial losses are summed on the host.

v3 structure:
  - alpha is host-transposed into 8 node-slabs (slab i = nodes
    [8i,8i+8) of all 1024 stages; each partition reads one contiguous
    32KB chunk) and streamed fp32 over the sync HWDGE queue; the last
    slab is split into 4 quarter-DMAs so its processing tail is short.
  - after slab i is reduced to softmax weights, DP steps 8i+2..8i+9
    run over all 8 stage-tiles, so the sequential scan streams along
    with the DMA instead of trailing it.
  - beta is cast fp32->bf16 during gpsimd SWDGE DMAs, paced one tile
    per slab by the gpsimd program order so it steals little alpha
    bandwidth; exp(beta) lands in one persistent SBUF buffer.
  - the incidence reduction runs with mt as the (shared) PE weights:
    per chunk one LDWEIGHTS feeds a 512-column matmul over 4 stage-
    tiles, accumulating c^T [67, 512] in PSUM (2 quad passes).  c^T is
    transposed back per-tile by PE transposes, then theta/denominator
    are folded into c before the DP finishes, leaving one fused
    dot + partition reduction as the only post-DP work.
"""

import numpy as np

SW = 16          # DP window
NN = 64          # nodes per stage
NOPS = 8         # ops per edge (last excluded from the max)
S = 8192         # stages
E = 2016         # beta edges
P = 128          # SBUF partitions
N_CORES = 8
S_CORE = S // N_CORES        # 1024
T = S_CORE // P              # 8 stage-tiles per core
NSLAB = 8                    # node slabs
NPS = NN // NSLAB            # 8 nodes per slab
SLABW = T * NPS * SW         # 1024 mx elems per slab (per partition)
AWS_ = SLABW * NOPS          # 8192 alpha floats per slab per partition
NQ = 4                       # quarters for the last slab
QW = AWS_ // NQ              # 2048 floats per quarter
EDW = 67                     # ed row stride (66 node slots + 1 pad)
NCH = 16                     # beta column chunks
ECH = E // NCH               # 126 edges per chunk
MNEG = -40.0                 # additive mask for invalid window rows

_CACHE = {}


def _host_consts():
    import ml_dtypes

    ii, jj = [], []
    for i in range(2, NN + 1):
        for j in range(i + 1, NN + 2):
            ii.append(i)
            jj.append(j)
    ii = np.asarray(ii)
    jj = np.asarray(jj)
    # incidence matrix chunks: mt[e_local, c*67 + k] = [ii==k] + [jj==k],
    # column 66 of each chunk is all ones (softmax denominator)
    mt = np.zeros((NCH, ECH, EDW), np.float32)
    for e in range(E):
        c, el = divmod(e, ECH)
        mt[c, el, ii[e]] += 1.0
        mt[c, el, jj[e]] += 1.0
        mt[c, el, EDW - 1] = 1.0
    mt = np.ascontiguousarray(
        mt.transpose(1, 0, 2).reshape(ECH, NCH * EDW)
    ).astype(ml_dtypes.bfloat16)

    # additive masks for slabs 0/1 (node n: rows k < n+2 valid);
    # slab layout is node-major [nl, t, k]
    def neg(nodes):
        m = np.zeros((len(nodes), T, SW), np.float32)
        for r, n in enumerate(nodes):
            m[r, :, min(n + 2, SW):] = MNEG
        return np.ascontiguousarray(
            np.broadcast_to(m.reshape(1, -1), (P, m.size))
        ).astype(ml_dtypes.bfloat16)

    mneg0 = neg(range(0, NPS))          # slab 0: nodes 0-7   [P, 1024]
    mneg1 = neg(range(NPS, 14))         # slab 1: nodes 8-13  [P, 768]
    ident = np.eye(P, dtype=np.float32)
    return mt, mneg0, mneg1, ident


def _build_nc():
    import concourse.bass as bass
    import concourse.mybir as mybir
    from concourse.tile import TileContext
    from concourse.vector_clock import ScopedClock, VectorClock

    # This walrus build rejects TPB instructions carrying more than one sem
    # wait (two for EventSemaphore), but Tile's wait assignment happily packs
    # 2-3. Split the extras onto single-wait NoOps on the same engine.
    if not getattr(TileContext, "_ant_wait_split", False):
        _orig_commit = TileContext._commit_instruction

        def _commit_split(self, inst, lazy_reg_writes=True):
            si = inst.sync_info
            limit = 2 if isinstance(inst, mybir.InstEventSemaphore) else 1
            if si is not None and si.on_wait and len(si.on_wait) > limit:
                waits = list(si.on_wait)
                for i, w in enumerate(waits[:-limit]):
                    nop = mybir.InstNoOp(
                        name=f"{inst.name}-sw{i}",
                        sync_info=mybir.SyncInfo(on_wait=[w], on_update=[]),
                        bass_nofuse=True,
                        engine=inst.engine,
                    )
                    _orig_commit(self, nop, lazy_reg_writes)
                inst.sync_info = mybir.SyncInfo(
                    on_wait=waits[-limit:], on_update=list(si.on_update)
                )
            return _orig_commit(self, inst, lazy_reg_writes)

        TileContext._commit_instruction = _commit_split
        TileContext._ant_wait_split = True

    # The stock TileContext tail drain packs every outstanding sem wait into
    # a single InstDrain; this walrus caps non-EventSemaphore instructions at
    # one wait. Emit one drain per outstanding semaphore instead.
    def _drain_and_barrier(self, tick_clock, wait_clock):
        nc = self.nc
        gc = tick_clock.global_clock
        n = len(gc)
        for i in range(n):
            t = gc[i]
            if t <= 0:
                continue
            vc = VectorClock([0] * n)
            vc.require_at_least(i, t)
            d = nc.sync.drain()
            wait_clock.add_sem_waits(d.ins, ScopedClock({None: vc}))
        nc.all_engine_barrier()
        assert self.sems is not None
        popped = nc._tile_sem_poison_stack.pop()
        assert popped is self._sem_poison
        nc.clear_and_free_semaphores(list(self.sems.allocated().values()))
        nc.all_engine_barrier()

    TileContext._drain_and_barrier = _drain_and_barrier

    f32 = mybir.dt.float32
    bf16 = mybir.dt.bfloat16
    Alu = mybir.AluOpType
    Act = mybir.ActivationFunctionType
    X = mybir.AxisListType.X

    nc = bass.Bass()
    # alpha host-transposed to slab-major: [slab, p, t, nl, k, o] flattened
    # to [NSLAB*P, 8192]; partition p's slab read is 32KB contiguous
    alpha_d = nc.declare_dram_parameter(
        "alpha_s", [NSLAB * P, AWS_], f32, isOutput=False
    )
    # beta pre-transposed on the host into chunk layout:
    # beta_t[el, t*2048 + c*128 + p] = beta[t*128 + p, c*126 + el]
    beta_d = nc.declare_dram_parameter("beta_t", [ECH, T * NCH * P], f32, isOutput=False)
    theta_d = nc.declare_dram_parameter("theta_t", [P, T], f32, isOutput=False)
    mneg0_d = nc.declare_dram_parameter("mneg0", [P, NPS * T * SW], bf16, isOutput=False)
    mneg1_d = nc.declare_dram_parameter("mneg1", [P, 6 * T * SW], bf16, isOutput=False)
    mt_d = nc.declare_dram_parameter("mt_c", [ECH, NCH * EDW], bf16, isOutput=False)
    ident_d = nc.declare_dram_parameter("ident", [P, P], f32, isOutput=False)
    out_d = nc.declare_dram_parameter("loss_part", [1, 1], f32, isOutput=True)

    with TileContext(nc) as tc:
        with (
            tc.tile_pool(name="consts", bufs=1) as cp,
            tc.tile_pool(name="alphap", bufs=3) as ap_pool,
            tc.tile_pool(name="mxp", bufs=2) as mxp,
            tc.tile_pool(name="persist", bufs=1) as pp,
            tc.tile_pool(name="smallp", bufs=4) as sp,
            tc.tile_pool(name="betap", bufs=4) as bp,
            tc.tile_pool(name="psc", bufs=2, space="PSUM") as psc,
        ):
            # consts first (small; bf16 masks) so alpha starts right after
            mneg0_sb = cp.tile([P, NPS * T * SW], bf16)
            nc.sync.dma_start(mneg0_sb[:, :], mneg0_d[:, :])
            mneg1_sb = cp.tile([P, 6 * T * SW], bf16)
            nc.sync.dma_start(mneg1_sb[:, :], mneg1_d[:, :])
            mt_sb = cp.tile([ECH, NCH * EDW], bf16)
            nc.sync.dma_start(mt_sb[:, :], mt_d[:, :])
            theta_sb = cp.tile([P, T], f32)
            nc.sync.dma_start(theta_sb[:, :], theta_d[:, :])
            ident_sb = cp.tile([P, P], f32)
            nc.sync.dma_start(ident_sb[:, :], ident_d[:, :])

            # ---- alpha DMAs on the sync HWDGE queue; slabs 0 and 7 are
            # quarter-split (0: earlier compute start; 7: short tail) ----
            a_sl = []
            for i in range(NSLAB):
                a = ap_pool.tile([P, AWS_], f32, tag="a")
                if i in (0, NSLAB - 1):
                    for q in range(NQ):
                        nc.sync.dma_start(
                            a[:, q * QW : (q + 1) * QW],
                            alpha_d[i * P : (i + 1) * P, q * QW : (q + 1) * QW],
                        )
                else:
                    nc.sync.dma_start(a[:, :], alpha_d[i * P : (i + 1) * P, :])
                a_sl.append(a)
            ones_sb = cp.tile([P, 1], f32)
            nc.vector.memset(ones_sb[:, :], 1.0)

            w_sb = pp.tile([P, NSLAB * SLABW], f32)  # weights, slab-major
            ed_sb = pp.tile([P, T * EDW], f32)       # DP state, zero-init
            tmp_sb = pp.tile([P, T * (SW + 1)], f32) # DP scratch; col 0 == 1.0
            eb_sb = pp.tile([ECH, T * NCH * P], bf16)  # exp(beta), all tiles
            ct_sb = pp.tile([EDW, T * P], f32)       # c^T quads (PSUM copies)
            c_sb = pp.tile([P, T * EDW], f32)        # c per stage-partition
            cw_sb = pp.tile([P, T * (EDW - 1)], f32) # c * theta / den
            nc.vector.memset(ed_sb[:, :], 0.0)

            ed3 = ed_sb.rearrange("p (t k) -> p t k", t=T)
            tmp3 = tmp_sb.rearrange("p (t k) -> p t k", k=SW + 1)
            nc.vector.memset(tmp3[:, :, 0:1], 1.0)
            w5 = w_sb.rearrange(
                "p (s n t k) -> p s n t k", s=NSLAB, n=NPS, k=SW
            )

            def dp_steps(j_lo, j_hi, fill=()):
                """DP steps with the multiply on gpsimd and the grouped
                reduce on DVE; `fill` supplies independent DVE thunks that
                are interleaved so DVE stays busy during the gp multiply."""
                fill = list(fill)
                for j in range(j_lo, j_hi):
                    n = j - 2
                    wid = min(j, SW)
                    base = j - wid
                    # sum(w) == 1, so ed[j] = 1 + sum(w * ed); the constant
                    # 1.0 lives in tmp col 0 and rides along in the reduce.
                    # Products of the settled window prefix run on gpsimd a
                    # full step ahead; DVE multiplies only ed[j-1].
                    nc.gpsimd.tensor_mul(
                        tmp3[:, :, 1:wid],
                        ed3[:, :, base : j - 1],
                        w5[:, n // NPS, n % NPS, :, 0 : wid - 1],
                    )
                    nc.vector.tensor_mul(
                        tmp3[:, :, wid : wid + 1],
                        ed3[:, :, j - 1 : j],
                        w5[:, n // NPS, n % NPS, :, wid - 1 : wid],
                    )
                    if fill:
                        fill.pop(0)()
                    nc.vector.reduce_sum(
                        ed3[:, :, j : j + 1], tmp3[:, :, 0 : wid + 1], axis=X
                    )
                for f in fill:
                    f()

            def slab_front(i, a_view, ncols, coff, nchunk=1):
                """Thunks for max-reduce + mask + exp + sums of `ncols` mx
                columns of slab i at column offset coff, split into `nchunk`
                pieces.  Returns (dve_thunks, norm_fn); the thunks interleave
                with a DP burst to keep DVE busy during the gp multiplies,
                and norm_fn is emitted after the burst so it never stalls
                the burst's gp stream."""
                a3 = a_view.rearrange("p (g o) -> p g o", o=NOPS)
                mx = mxp.tile([P, SLABW], f32, tag="mx", bufs=2)
                mxc = mx[:, coff : coff + ncols]
                e_sl = w_sb[:, i * SLABW + coff : i * SLABW + coff + ncols]
                ng = ncols // SW
                ngc = ng // nchunk
                gper = ngc * SW
                s_t = sp.tile([P, T * NPS], f32, tag="s")
                rs = sp.tile([P, T * NPS], f32, tag="rs")
                mn = (mneg0_sb if i == 0 else mneg1_sb) if i < 2 else None

                def front(c):
                    nc.vector.reduce_max(
                        mxc[:, c * gper : (c + 1) * gper],
                        a3[:, c * gper : (c + 1) * gper, 0 : NOPS - 1],
                        axis=X,
                    )
                    if mn is not None:
                        lo, hi = c * gper, min((c + 1) * gper, mn.shape[1])
                        if lo < hi:
                            nc.vector.tensor_add(
                                mx[:, lo:hi], mx[:, lo:hi], mn[:, lo:hi]
                            )
                    nc.scalar.activation(
                        e_sl[:, c * gper : (c + 1) * gper],
                        mxc[:, c * gper : (c + 1) * gper],
                        Act.Exp,
                    )

                def sums(c):
                    nc.vector.reduce_sum(
                        s_t[:, c * ngc : (c + 1) * ngc],
                        e_sl[:, c * gper : (c + 1) * gper].rearrange(
                            "p (n k) -> p n k", k=SW
                        ),
                        axis=X,
                    )
                    if c == nchunk - 1:
                        nc.vector.reciprocal(rs[:, 0:ng], s_t[:, 0:ng])

                thunks = [lambda c=c: front(c) for c in range(nchunk)]
                thunks += [lambda c=c: sums(c) for c in range(nchunk)]

                def norm_fn():
                    rs_b = rs[:, 0:ng].rearrange(
                        "p (n o) -> p n o", o=1
                    ).broadcast_to((P, ng, SW))
                    e3 = e_sl.rearrange("p (n k) -> p n k", k=SW)
                    nc.gpsimd.tensor_mul(e3, e3, rs_b)

                return thunks, norm_fn

            # beta incidence matmuls: mt as shared weights, quad passes.
            # quad h covers stage-tiles 4h..4h+3; out is c^T [67, 512].
            ebv = eb_sb.rearrange("e (t c p) -> e t c p", t=T, p=P)
            ct_ps = psc.tile([P, T * P], f32, tag="ct", bufs=1)

            def beta_quad(h):
                cps = psc.tile([EDW, 4 * P], f32, tag="cq")
                for c in range(NCH):
                    nc.tensor.matmul(
                        cps[:, :],
                        mt_sb[:, c * EDW : (c + 1) * EDW],
                        ebv[:, 4 * h : 4 * h + 4, c, :],
                        start=(c == 0),
                        stop=(c == NCH - 1),
                    )
                nc.scalar.copy(ct_sb[:, h * 4 * P : (h + 1) * 4 * P], cps[:, :])
                for t in range(4 * h, 4 * h + 4):
                    nc.tensor.transpose(
                        ct_ps[:, t * P : t * P + EDW],
                        ct_sb[:, t * P : (t + 1) * P],
                        ident_sb[0:EDW, 0:EDW],
                    )
                    nc.scalar.copy(
                        c_sb[:, t * EDW : (t + 1) * EDW],
                        ct_ps[:, t * P : t * P + EDW],
                    )

            # beta DMA triggers paced ~one per slab on the gp SWDGE queue;
            # exp(beta) trails its DMA by one slab so ACT never stalls on it
            B_TRIG = {0: (0, 1), 1: (2,), 2: (3,), 3: (4,), 4: (5,), 5: (6, 7)}
            B_EXP = {1: (0, 1), 2: (2,), 3: (3,), 4: (4,), 5: (5,), 6: (6, 7)}
            b_t = {}

            # ---- slab loop (slabs 0-6 whole; slab 7 per quarter) ----
            # tile_wait_until pins each region's scheduler dispatch time so
            # the list scheduler cannot hoist late-phase ops (whose sems
            # would then head-block an engine's in-order stream)
            for i in range(NSLAB - 1):
                with tc.tile_wait_until(0.012 + 0.0122 * i):
                    th, norm_fn = slab_front(
                        i, a_sl[i], SLABW, 0, nchunk=(4 if i == 0 else 1)
                    )
                    for f in th:
                        f()
                    if i > 0:
                        dp_steps(2 + (i - 1) * NPS, 2 + i * NPS)
                    norm_fn()
                    for t in B_TRIG.get(i, ()):
                        b = bp.tile([ECH, NCH * P], bf16, tag="b")
                        nc.gpsimd.dma_start(
                            b[:, :], beta_d[:, t * NCH * P : (t + 1) * NCH * P]
                        )
                        b_t[t] = b
                    for t in B_EXP.get(i, ()):
                        nc.scalar.activation(
                            eb_sb[:, t * NCH * P : (t + 1) * NCH * P],
                            b_t.pop(t)[:, :],
                            Act.Exp,
                        )
                    if i == 4:
                        beta_quad(0)

            # slab 7 quarters: 2 nodes each; DP trails by one quarter
            i7 = NSLAB - 1
            a7 = a_sl[i7]
            for q in range(NQ):
                with tc.tile_wait_until(0.102 + 0.004 * q):
                    qv = a7[:, q * QW : (q + 1) * QW]
                    th, norm_fn = slab_front(
                        i7, qv, QW // NOPS, q * (SLABW // NQ)
                    )
                    for f in th:
                        f()
                    if q == 0:
                        dp_steps(2 + (i7 - 1) * NPS, 2 + i7 * NPS)
                        norm_fn()
                        beta_quad(1)
                    else:
                        dp_steps(
                            2 + i7 * NPS + (q - 1) * 2, 2 + i7 * NPS + q * 2
                        )
                        norm_fn()

            # fold theta/denominator into c while the DP finishes
            with tc.tile_wait_until(0.124):
                c3 = c_sb.rearrange("p (t k) -> p t k", t=T)
                den = sp.tile([P, T], f32, tag="den")
                nc.vector.reciprocal(den[:, :], c3[:, :, EDW - 1])
                thr = sp.tile([P, T], f32, tag="thr")
                nc.vector.tensor_mul(thr[:, :], den[:, :], theta_sb[:, :])
                cw3 = cw_sb.rearrange("p (t k) -> p t k", t=T)
                nc.vector.tensor_mul(
                    cw3,
                    c3[:, :, 0 : EDW - 1],
                    thr.rearrange("p (t o) -> p t o", o=1).broadcast_to(
                        (P, T, EDW - 1)
                    ),
                )
                dp_steps(NN, NN + 2)

            # ---- post-DP: one fused dot + partition reduction ----
            with tc.tile_wait_until(0.127):
                prod = pp.tile([P, T * (EDW - 1)], f32)
                lrow = sp.tile([P, 1], f32, tag="lrow")
                nc.vector.scalar_tensor_tensor(
                    prod[:, :],
                    ed3[:, :, 0 : EDW - 1],
                    0.0,
                    cw3,
                    Alu.add,
                    Alu.mult,
                    accum_out=lrow[:, :],
                )
                out_ps = psc.tile([1, 1], f32, tag="outp", bufs=1)
                nc.tensor.matmul(
                    out_ps[:, :], lrow[:, :], ones_sb[:, :], start=True, stop=True
                )
                out_sb = sp.tile([1, 1], f32, tag="outs")
                nc.scalar.copy(out_sb[:, :], out_ps[:, :])
                nc.sync.dma_start(out_d[:, :], out_sb[:, :])

    return nc


def _get_compiled():
    if "nc" not in _CACHE:
        _CACHE["nc"] = _build_nc()
        _CACHE["consts"] = _host_consts()
    return _CACHE["nc"], _CACHE["consts"]


def _in_maps(alpha, beta, theta):
    mt, mneg0, mneg1, ident = _get_compiled()[1]
    alpha = np.ascontiguousarray(alpha, dtype=np.float32).reshape(S, NN * SW * NOPS)
    beta = np.ascontiguousarray(beta, dtype=np.float32)
    theta = np.ascontiguousarray(theta, dtype=np.float32)
    maps = []
    for c in range(N_CORES):
        sl = slice(c * S_CORE, (c + 1) * S_CORE)
        # [slab, p, nl, t, k*o]: partition p reads 32KB contiguous per
        # slab; node-major within the slab so quarter-DMAs cover node
        # prefixes for all stage-tiles
        alpha_s = np.ascontiguousarray(
            alpha[sl]
            .reshape(T, P, NSLAB, NPS, SW * NOPS)
            .transpose(2, 1, 3, 0, 4)
            .reshape(NSLAB * P, AWS_)
        )
        # [el, t*2048 + ch*128 + p] = beta[t*128 + p, ch*126 + el]
        beta_t = np.ascontiguousarray(
            beta[sl].reshape(T, P, NCH, ECH).transpose(3, 0, 2, 1).reshape(ECH, -1)
        )
        maps.append(
            {
                "alpha_s": alpha_s,
                "beta_t": beta_t,
                "theta_t": np.ascontiguousarray(theta[sl].reshape(T, P).T),
                "mneg0": mneg0,
                "mneg1": mneg1,
                "mt_c": mt,
                "ident": ident,
            }
        )
    return maps


def _run(alpha, beta, theta, **spmd_kwargs):
    from concourse.bass_utils import run_bass_kernel_spmd

    nc, _ = _get_compiled()
    res = run_bass_kernel_spmd(
        nc, _in_maps(alpha, beta, theta), core_ids=list(range(N_CORES)), **spmd_kwargs
    )
    total = np.float32(0.0)
    for r in res.results:
        total += np.float32(r["loss_part"][0, 0])
    return np.float32(total), res


def kernel(alpha, beta, theta):
    out, _ = _run(alpha, beta, theta)
    return out


# revision 32
# speedup vs baseline: 1.1177x; 1.0419x over previous
"""Trainium2 Bass kernel for the Expected-Depth DP loss.

Computation (see reference):
  - edge_max = max over first 7 of 8 op-logits          [S, 64, 16]
  - w        = masked softmax over the 16-wide window   [S, 64, 16]
  - DP scan:  ed[j] = sum_k w[j,k] * (ed[base+k] + 1),  j = 2..65
  - loss     = sum_s theta[s] * softmax(beta[s]) . (ed[ii] + ed[jj])

Sharding: S=8192 stages split across 8 cores (pure data parallel,
1024 stages/core). Per-core partial losses are summed on the host.

v3 structure:
  - alpha is host-transposed into 8 node-slabs (slab i = nodes
    [8i,8i+8) of all 1024 stages; each partition reads one contiguous
    32KB chunk) and streamed fp32 over the sync HWDGE queue; the last
    slab is split into 4 quarter-DMAs so its processing tail is short.
  - after slab i is reduced to softmax weights, DP steps 8i+2..8i+9
    run over all 8 stage-tiles, so the sequential scan streams along
    with the DMA instead of trailing it.
  - beta is cast fp32->bf16 during gpsimd SWDGE DMAs, paced one tile
    per slab by the gpsimd program order so it steals little alpha
    bandwidth; exp(beta) lands in one persistent SBUF buffer.
  - the incidence reduction runs with mt as the (shared) PE weights:
    per chunk one LDWEIGHTS feeds a 512-column matmul over 4 stage-
    tiles, accumulating c^T [67, 512] in PSUM (2 quad passes).  c^T is
    transposed back per-tile by PE transposes, then theta/denominator
    are folded into c before the DP finishes, leaving one fused
    dot + partition reduction as the only post-DP work.
"""

import numpy as np

SW = 16          # DP window
NN = 64          # nodes per stage
NOPS = 8         # ops per edge (last excluded from the max)
S = 8192         # stages
E = 2016         # beta edges
P = 128          # SBUF partitions
N_CORES = 8
S_CORE = S // N_CORES        # 1024
T = S_CORE // P              # 8 stage-tiles per core
NSLAB = 8                    # node slabs
NPS = NN // NSLAB            # 8 nodes per slab
SLABW = T * NPS * SW         # 1024 mx elems per slab (per partition)
AWS_ = SLABW * NOPS          # 8192 alpha floats per slab per partition
NQ = 4                       # quarters for the last slab
QW = AWS_ // NQ              # 2048 floats per quarter
EDW = 67                     # ed row stride (66 node slots + 1 pad)
NCH = 16                     # beta column chunks
ECH = E // NCH               # 126 edges per chunk
MNEG = -40.0                 # additive mask for invalid window rows

_CACHE = {}


def _host_consts():
    import ml_dtypes

    ii, jj = [], []
    for i in range(2, NN + 1):
        for j in range(i + 1, NN + 2):
            ii.append(i)
            jj.append(j)
    ii = np.asarray(ii)
    jj = np.asarray(jj)
    # incidence matrix chunks: mt[e_local, c*67 + k] = [ii==k] + [jj==k],
    # column 66 of each chunk is all ones (softmax denominator)
    mt = np.zeros((NCH, ECH, EDW), np.float32)
    for e in range(E):
        c, el = divmod(e, ECH)
        mt[c, el, ii[e]] += 1.0
        mt[c, el, jj[e]] += 1.0
        mt[c, el, EDW - 1] = 1.0
    mt = np.ascontiguousarray(
        mt.transpose(1, 0, 2).reshape(ECH, NCH * EDW)
    ).astype(ml_dtypes.bfloat16)

    # additive masks for slabs 0/1 (node n: rows k < n+2 valid);
    # slab layout is node-major [nl, t, k]
    def neg(nodes):
        m = np.zeros((len(nodes), T, SW), np.float32)
        for r, n in enumerate(nodes):
            m[r, :, min(n + 2, SW):] = MNEG
        return np.ascontiguousarray(
            np.broadcast_to(m.reshape(1, -1), (P, m.size))
        ).astype(ml_dtypes.bfloat16)

    mneg0 = neg(range(0, NPS))          # slab 0: nodes 0-7   [P, 1024]
    mneg1 = neg(range(NPS, 14))         # slab 1: nodes 8-13  [P, 768]
    ident = np.eye(P, dtype=np.float32)
    return mt, mneg0, mneg1, ident


def _build_nc():
    import concourse.bass as bass
    import concourse.mybir as mybir
    from concourse.tile import TileContext
    from concourse.vector_clock import ScopedClock, VectorClock

    # This walrus build rejects TPB instructions carrying more than one sem
    # wait (two for EventSemaphore), but Tile's wait assignment happily packs
    # 2-3. Split the extras onto single-wait NoOps on the same engine.
    if not getattr(TileContext, "_ant_wait_split", False):
        _orig_commit = TileContext._commit_instruction

        def _commit_split(self, inst, lazy_reg_writes=True):
            si = inst.sync_info
            limit = 2 if isinstance(inst, mybir.InstEventSemaphore) else 1
            if si is not None and si.on_wait and len(si.on_wait) > limit:
                waits = list(si.on_wait)
                for i, w in enumerate(waits[:-limit]):
                    nop = mybir.InstNoOp(
                        name=f"{inst.name}-sw{i}",
                        sync_info=mybir.SyncInfo(on_wait=[w], on_update=[]),
                        bass_nofuse=True,
                        engine=inst.engine,
                    )
                    _orig_commit(self, nop, lazy_reg_writes)
                inst.sync_info = mybir.SyncInfo(
                    on_wait=waits[-limit:], on_update=list(si.on_update)
                )
            return _orig_commit(self, inst, lazy_reg_writes)

        TileContext._commit_instruction = _commit_split
        TileContext._ant_wait_split = True

    # The stock TileContext tail drain packs every outstanding sem wait into
    # a single InstDrain; this walrus caps non-EventSemaphore instructions at
    # one wait. Emit one drain per outstanding semaphore instead.
    def _drain_and_barrier(self, tick_clock, wait_clock):
        nc = self.nc
        gc = tick_clock.global_clock
        n = len(gc)
        for i in range(n):
            t = gc[i]
            if t <= 0:
                continue
            vc = VectorClock([0] * n)
            vc.require_at_least(i, t)
            d = nc.sync.drain()
            wait_clock.add_sem_waits(d.ins, ScopedClock({None: vc}))
        nc.all_engine_barrier()
        assert self.sems is not None
        popped = nc._tile_sem_poison_stack.pop()
        assert popped is self._sem_poison
        nc.clear_and_free_semaphores(list(self.sems.allocated().values()))
        nc.all_engine_barrier()

    TileContext._drain_and_barrier = _drain_and_barrier

    f32 = mybir.dt.float32
    bf16 = mybir.dt.bfloat16
    Alu = mybir.AluOpType
    Act = mybir.ActivationFunctionType
    X = mybir.AxisListType.X

    nc = bass.Bass()
    # alpha host-transposed to slab-major: [slab, p, t, nl, k, o] flattened
    # to [NSLAB*P, 8192]; partition p's slab read is 32KB contiguous
    alpha_d = nc.declare_dram_parameter(
        "alpha_s", [NSLAB * P, AWS_], f32, isOutput=False
    )
    # beta pre-transposed on the host into chunk layout:
    # beta_t[el, t*2048 + c*128 + p] = beta[t*128 + p, c*126 + el]
    beta_d = nc.declare_dram_parameter("beta_t", [ECH, T * NCH * P], f32, isOutput=False)
    theta_d = nc.declare_dram_parameter("theta_t", [P, T], f32, isOutput=False)
    mneg0_d = nc.declare_dram_parameter("mneg0", [P, NPS * T * SW], bf16, isOutput=False)
    mneg1_d = nc.declare_dram_parameter("mneg1", [P, 6 * T * SW], bf16, isOutput=False)
    mt_d = nc.declare_dram_parameter("mt_c", [ECH, NCH * EDW], bf16, isOutput=False)
    ident_d = nc.declare_dram_parameter("ident", [P, P], f32, isOutput=False)
    out_d = nc.declare_dram_parameter("loss_part", [1, 1], f32, isOutput=True)

    with TileContext(nc) as tc:
        with (
            tc.tile_pool(name="consts", bufs=1) as cp,
            tc.tile_pool(name="alphap", bufs=3) as ap_pool,
            tc.tile_pool(name="mxp", bufs=2) as mxp,
            tc.tile_pool(name="persist", bufs=1) as pp,
            tc.tile_pool(name="smallp", bufs=4) as sp,
            tc.tile_pool(name="betap", bufs=4) as bp,
            tc.tile_pool(name="psc", bufs=2, space="PSUM") as psc,
        ):
            # consts first (small; bf16 masks) so alpha starts right after
            mneg0_sb = cp.tile([P, NPS * T * SW], bf16)
            nc.sync.dma_start(mneg0_sb[:, :], mneg0_d[:, :])
            mneg1_sb = cp.tile([P, 6 * T * SW], bf16)
            nc.sync.dma_start(mneg1_sb[:, :], mneg1_d[:, :])
            mt_sb = cp.tile([ECH, NCH * EDW], bf16)
            nc.sync.dma_start(mt_sb[:, :], mt_d[:, :])
            theta_sb = cp.tile([P, T], f32)
            nc.sync.dma_start(theta_sb[:, :], theta_d[:, :])
            ident_sb = cp.tile([P, P], f32)
            nc.sync.dma_start(ident_sb[:, :], ident_d[:, :])

            # ---- alpha DMAs on the sync HWDGE queue; slabs 0 and 7 are
            # quarter-split (0: earlier compute start; 7: short tail) ----
            a_sl = []
            for i in range(NSLAB):
                a = ap_pool.tile([P, AWS_], f32, tag="a")
                if i in (0, NSLAB - 1):
                    for q in range(NQ):
                        nc.sync.dma_start(
                            a[:, q * QW : (q + 1) * QW],
                            alpha_d[i * P : (i + 1) * P, q * QW : (q + 1) * QW],
                        )
                else:
                    nc.sync.dma_start(a[:, :], alpha_d[i * P : (i + 1) * P, :])
                a_sl.append(a)
            ones_sb = cp.tile([P, 1], f32)
            nc.vector.memset(ones_sb[:, :], 1.0)

            w_sb = pp.tile([P, NSLAB * SLABW], f32)  # weights, slab-major
            ed_sb = pp.tile([P, T * EDW], f32)       # DP state, zero-init
            tmp_sb = pp.tile([P, T * (SW + 1)], f32) # DP scratch; col 0 == 1.0
            eb_sb = pp.tile([ECH, T * NCH * P], bf16)  # exp(beta), all tiles
            ct_sb = pp.tile([EDW, T * P], f32)       # c^T quads (PSUM copies)
            c_sb = pp.tile([P, T * EDW], f32)        # c per stage-partition
            cw_sb = pp.tile([P, T * (EDW - 1)], f32) # c * theta / den
            nc.vector.memset(ed_sb[:, :], 0.0)

            ed3 = ed_sb.rearrange("p (t k) -> p t k", t=T)
            tmp3 = tmp_sb.rearrange("p (t k) -> p t k", k=SW + 1)
            nc.vector.memset(tmp3[:, :, 0:1], 1.0)
            w5 = w_sb.rearrange(
                "p (s n t k) -> p s n t k", s=NSLAB, n=NPS, k=SW
            )

            def dp_steps(j_lo, j_hi, fill=()):
                """DP steps with the multiply on gpsimd and the grouped
                reduce on DVE; `fill` supplies independent DVE thunks that
                are interleaved so DVE stays busy during the gp multiply."""
                fill = list(fill)
                for j in range(j_lo, j_hi):
                    n = j - 2
                    wid = min(j, SW)
                    base = j - wid
                    # sum(w) == 1, so ed[j] = 1 + sum(w * ed); the constant
                    # 1.0 lives in tmp col 0 and rides along in the reduce.
                    nc.vector.tensor_mul(
                        tmp3[:, :, 1 : wid + 1],
                        ed3[:, :, base : base + wid],
                        w5[:, n // NPS, n % NPS, :, 0:wid],
                    )
                    if fill:
                        fill.pop(0)()
                    nc.vector.reduce_sum(
                        ed3[:, :, j : j + 1], tmp3[:, :, 0 : wid + 1], axis=X
                    )
                for f in fill:
                    f()

            def slab_front(i, a_view, ncols, coff, nchunk=1):
                """Thunks for max-reduce + mask + exp + sums of `ncols` mx
                columns of slab i at column offset coff, split into `nchunk`
                pieces.  Returns (dve_thunks, norm_fn); the thunks interleave
                with a DP burst to keep DVE busy during the gp multiplies,
                and norm_fn is emitted after the burst so it never stalls
                the burst's gp stream."""
                a3 = a_view.rearrange("p (g o) -> p g o", o=NOPS)
                mx = mxp.tile([P, SLABW], f32, tag="mx", bufs=2)
                mxc = mx[:, coff : coff + ncols]
                e_sl = w_sb[:, i * SLABW + coff : i * SLABW + coff + ncols]
                ng = ncols // SW
                ngc = ng // nchunk
                gper = ngc * SW
                s_t = sp.tile([P, T * NPS], f32, tag="s")
                rs = sp.tile([P, T * NPS], f32, tag="rs")
                mn = (mneg0_sb if i == 0 else mneg1_sb) if i < 2 else None

                def front(c):
                    nc.vector.reduce_max(
                        mxc[:, c * gper : (c + 1) * gper],
                        a3[:, c * gper : (c + 1) * gper, 0 : NOPS - 1],
                        axis=X,
                    )
                    if mn is not None:
                        lo, hi = c * gper, min((c + 1) * gper, mn.shape[1])
                        if lo < hi:
                            nc.vector.tensor_add(
                                mx[:, lo:hi], mx[:, lo:hi], mn[:, lo:hi]
                            )
                    nc.scalar.activation(
                        e_sl[:, c * gper : (c + 1) * gper],
                        mxc[:, c * gper : (c + 1) * gper],
                        Act.Exp,
                    )

                def sums(c):
                    nc.vector.reduce_sum(
                        s_t[:, c * ngc : (c + 1) * ngc],
                        e_sl[:, c * gper : (c + 1) * gper].rearrange(
                            "p (n k) -> p n k", k=SW
                        ),
                        axis=X,
                    )
                    if c == nchunk - 1:
                        nc.vector.reciprocal(rs[:, 0:ng], s_t[:, 0:ng])

                thunks = [lambda c=c: front(c) for c in range(nchunk)]
                thunks += [lambda c=c: sums(c) for c in range(nchunk)]

                def norm_fn():
                    rs_b = rs[:, 0:ng].rearrange(
                        "p (n o) -> p n o", o=1
                    ).broadcast_to((P, ng, SW))
                    e3 = e_sl.rearrange("p (n k) -> p n k", k=SW)
                    nc.gpsimd.tensor_mul(e3, e3, rs_b)

                return thunks, norm_fn

            # beta incidence matmuls: mt as shared weights, quad passes.
            # quad h covers stage-tiles 4h..4h+3; out is c^T [67, 512].
            ebv = eb_sb.rearrange("e (t c p) -> e t c p", t=T, p=P)
            ct_ps = psc.tile([P, T * P], f32, tag="ct", bufs=1)

            def beta_quad(h):
                cps = psc.tile([EDW, 4 * P], f32, tag="cq")
                for c in range(NCH):
                    nc.tensor.matmul(
                        cps[:, :],
                        mt_sb[:, c * EDW : (c + 1) * EDW],
                        ebv[:, 4 * h : 4 * h + 4, c, :],
                        start=(c == 0),
                        stop=(c == NCH - 1),
                    )
                nc.scalar.copy(ct_sb[:, h * 4 * P : (h + 1) * 4 * P], cps[:, :])
                for t in range(4 * h, 4 * h + 4):
                    nc.tensor.transpose(
                        ct_ps[:, t * P : t * P + EDW],
                        ct_sb[:, t * P : (t + 1) * P],
                        ident_sb[0:EDW, 0:EDW],
                    )
                    nc.scalar.copy(
                        c_sb[:, t * EDW : (t + 1) * EDW],
                        ct_ps[:, t * P : t * P + EDW],
                    )

            # beta DMA triggers paced ~one per slab on the gp SWDGE queue;
            # exp(beta) trails its DMA by one slab so ACT never stalls on it
            B_TRIG = {0: (0, 1), 1: (2,), 2: (3,), 3: (4,), 4: (5,), 5: (6, 7)}
            B_EXP = {1: (0, 1), 2: (2,), 3: (3,), 4: (4,), 5: (5,), 6: (6, 7)}
            b_t = {}

            # ---- slab loop (slabs 0-6 whole; slab 7 per quarter) ----
            # tile_wait_until pins each region's scheduler dispatch time so
            # the list scheduler cannot hoist late-phase ops (whose sems
            # would then head-block an engine's in-order stream)
            for i in range(NSLAB - 1):
                with tc.tile_wait_until(0.012 + 0.0122 * i):
                    th, norm_fn = slab_front(
                        i, a_sl[i], SLABW, 0, nchunk=(4 if i == 0 else 1)
                    )
                    for f in th:
                        f()
                    if i > 0:
                        dp_steps(2 + (i - 1) * NPS, 2 + i * NPS)
                    norm_fn()
                    for t in B_TRIG.get(i, ()):
                        b = bp.tile([ECH, NCH * P], bf16, tag="b")
                        nc.gpsimd.dma_start(
                            b[:, :], beta_d[:, t * NCH * P : (t + 1) * NCH * P]
                        )
                        b_t[t] = b
                    for t in B_EXP.get(i, ()):
                        nc.scalar.activation(
                            eb_sb[:, t * NCH * P : (t + 1) * NCH * P],
                            b_t.pop(t)[:, :],
                            Act.Exp,
                        )
                    if i == 4:
                        beta_quad(0)

            # slab 7 quarters: 2 nodes each; DP trails by one quarter
            i7 = NSLAB - 1
            a7 = a_sl[i7]
            for q in range(NQ):
                with tc.tile_wait_until(0.102 + 0.004 * q):
                    qv = a7[:, q * QW : (q + 1) * QW]
                    th, norm_fn = slab_front(
                        i7, qv, QW // NOPS, q * (SLABW // NQ)
                    )
                    for f in th:
                        f()
                    if q == 0:
                        dp_steps(2 + (i7 - 1) * NPS, 2 + i7 * NPS)
                        norm_fn()
                        beta_quad(1)
                    else:
                        dp_steps(
                            2 + i7 * NPS + (q - 1) * 2, 2 + i7 * NPS + q * 2
                        )
                        norm_fn()

            # fold theta/denominator into c while the DP finishes
            with tc.tile_wait_until(0.124):
                c3 = c_sb.rearrange("p (t k) -> p t k", t=T)
                den = sp.tile([P, T], f32, tag="den")
                nc.vector.reciprocal(den[:, :], c3[:, :, EDW - 1])
                thr = sp.tile([P, T], f32, tag="thr")
                nc.vector.tensor_mul(thr[:, :], den[:, :], theta_sb[:, :])
                cw3 = cw_sb.rearrange("p (t k) -> p t k", t=T)
                nc.vector.tensor_mul(
                    cw3,
                    c3[:, :, 0 : EDW - 1],
                    thr.rearrange("p (t o) -> p t o", o=1).broadcast_to(
                        (P, T, EDW - 1)
                    ),
                )
                dp_steps(NN, NN + 2)

            # ---- post-DP: one fused dot + partition reduction ----
            with tc.tile_wait_until(0.127):
                prod = pp.tile([P, T * (EDW - 1)], f32)
                lrow = sp.tile([P, 1], f32, tag="lrow")
                nc.vector.scalar_tensor_tensor(
                    prod[:, :],
                    ed3[:, :, 0 : EDW - 1],
                    0.0,
                    cw3,
                    Alu.add,
                    Alu.mult,
                    accum_out=lrow[:, :],
                )
                out_ps = psc.tile([1, 1], f32, tag="outp", bufs=1)
                nc.tensor.matmul(
                    out_ps[:, :], lrow[:, :], ones_sb[:, :], start=True, stop=True
                )
                out_sb = sp.tile([1, 1], f32, tag="outs")
                nc.scalar.copy(out_sb[:, :], out_ps[:, :])
                nc.sync.dma_start(out_d[:, :], out_sb[:, :])

    return nc


def _get_compiled():
    if "nc" not in _CACHE:
        _CACHE["nc"] = _build_nc()
        _CACHE["consts"] = _host_consts()
    return _CACHE["nc"], _CACHE["consts"]


def _in_maps(alpha, beta, theta):
    mt, mneg0, mneg1, ident = _get_compiled()[1]
    alpha = np.ascontiguousarray(alpha, dtype=np.float32).reshape(S, NN * SW * NOPS)
    beta = np.ascontiguousarray(beta, dtype=np.float32)
    theta = np.ascontiguousarray(theta, dtype=np.float32)
    maps = []
    for c in range(N_CORES):
        sl = slice(c * S_CORE, (c + 1) * S_CORE)
        # [slab, p, nl, t, k*o]: partition p reads 32KB contiguous per
        # slab; node-major within the slab so quarter-DMAs cover node
        # prefixes for all stage-tiles
        alpha_s = np.ascontiguousarray(
            alpha[sl]
            .reshape(T, P, NSLAB, NPS, SW * NOPS)
            .transpose(2, 1, 3, 0, 4)
            .reshape(NSLAB * P, AWS_)
        )
        # [el, t*2048 + ch*128 + p] = beta[t*128 + p, ch*126 + el]
        beta_t = np.ascontiguousarray(
            beta[sl].reshape(T, P, NCH, ECH).transpose(3, 0, 2, 1).reshape(ECH, -1)
        )
        maps.append(
            {
                "alpha_s": alpha_s,
                "beta_t": beta_t,
                "theta_t": np.ascontiguousarray(theta[sl].reshape(T, P).T),
                "mneg0": mneg0,
                "mneg1": mneg1,
                "mt_c": mt,
                "ident": ident,
            }
        )
    return maps


def _run(alpha, beta, theta, **spmd_kwargs):
    from concourse.bass_utils import run_bass_kernel_spmd

    nc, _ = _get_compiled()
    res = run_bass_kernel_spmd(
        nc, _in_maps(alpha, beta, theta), core_ids=list(range(N_CORES)), **spmd_kwargs
    )
    total = np.float32(0.0)
    for r in res.results:
        total += np.float32(r["loss_part"][0, 0])
    return np.float32(total), res


def kernel(alpha, beta, theta):
    out, _ = _run(alpha, beta, theta)
    return out
